# revision 38
# baseline (speedup 1.0000x reference)
"""Bahdanau-attention kernel for Trainium2 (8 NeuronCores, SPMD data parallel).

Math: the reference's per-step softmax is over a singleton axis, so the
attention weights are exactly 1.0. Hence:
    context  = values.sum(axis=1)            [B, DV]
    attn     = ones(B, T, 1)
    coverage[b, t, 0] = t                    [B, T, 1]
The W1/W2/W3/V MLP cancels out of every output.

Device work: per core, reduce a [B/8, T, DV] shard of `values` over T.
All chunks stream in via plain HWDGE loads; the fp32 adds are spread over
three engines so none exceeds the HBM/DMA roofline:
  - DVE: most chunks, serial tensor_add chain into dacc (plus merging gacc)
  - GpSimd: a few chunks, its own small add chain into gacc
  - PE: a few chunks matmul'd directly vs a ones column into the PSUM
    accumulation group, which also contracts dacc over partitions.
attn/coverage come from a tiny host const tensor, written out by DMA.
"""

import os
import numpy as np

B, T, DV = 32, 2048, 1024
NCORES = 8
BP = B // NCORES          # 4 batches per core
TCH = 128                 # t-chunk rows = SBUF partitions
NCH = T // TCH            # 16 chunks of [128, DV] per batch
NSPLIT = 512              # PSUM bank free-dim limit (f32)
NJ = DV // NSPLIT         # 2 psum column groups

# Engine roles (measured per-chunk costs: DVE add 1.22us, gpsimd add
# 2.4us, PE direct contraction ~2.5us = 4 MM insts):
#   - gpsimd: 5 early chunks per batch in its own chain (gacc)
#   - DVE: two short chains per batch (dacc_a over A_CHUNKS, dacc_b over
#     B_CHUNKS); gacc folds into dacc_a mid-stream, so dacc_a is ready by
#     ~chunk 11 and dacc_b one add after chunk 14
#   - PE: the two contractions plus chunk 15 direct, so the group's stop
#     matmul depends only on the final chunk's arrival (short tail)
GP_CHUNKS = (1, 4, 7, 10)
A_CHUNKS = (0, 2, 3, 5, 6)
B_CHUNKS = (8, 9, 11, 12, 13, 14, 15)
PE_CHUNK = None
GP_FOLD_AFTER = 13        # gacc -> dacc_a fold point (gacc done by ~k13)

_CACHE = {}
LAST = {}                 # exec_time_ns etc. for the test harness


def _build_nc():
    import concourse.tile as tile
    from concourse import bacc, mybir
    from contextlib import ExitStack

    f32 = mybir.dt.float32
    nc = bacc.Bacc(
        "TRN2", target_bir_lowering=False, debug=False, num_devices=NCORES
    )

    vals = nc.dram_tensor("vals", [BP, T, DV], f32, kind="ExternalInput").ap()
    consts = nc.dram_tensor("consts", [2, T], f32, kind="ExternalInput").ap()
    ctx_out = nc.dram_tensor("ctx_out", [BP, DV], f32, kind="ExternalOutput").ap()
    attn_out = nc.dram_tensor("attn_out", [BP, T, 1], f32, kind="ExternalOutput").ap()
    cov_out = nc.dram_tensor("cov_out", [BP, T, 1], f32, kind="ExternalOutput").ap()

    with tile.TileContext(nc) as tc, ExitStack() as ctx:
        cpool = ctx.enter_context(tc.tile_pool(name="const", bufs=1))
        vpool = ctx.enter_context(tc.tile_pool(name="vals", bufs=24))
        dpool = ctx.enter_context(tc.tile_pool(name="dacc", bufs=1))
        ppool = ctx.enter_context(tc.tile_pool(name="ps", bufs=1, space="PSUM"))
        opool = ctx.enter_context(tc.tile_pool(name="out", bufs=2))

        ones_t = cpool.tile([128, 1], f32)
        nc.vector.memset(ones_t[:], 1.0)

        const_t = cpool.tile([2, T], f32)
        nc.sync.dma_start(out=const_t[:], in_=consts[:])

        # attn/coverage writes go on the scalar HWDGE ring, issued as soon
        # as const_t lands, so the sync ring carries only the big loads.
        for b in range(BP):
            nc.scalar.dma_start(out=attn_out[b:b + 1, :, 0], in_=const_t[0:1, :])
            nc.scalar.dma_start(out=cov_out[b:b + 1, :, 0], in_=const_t[1:2, :])

        for b in range(BP):
            dacc_a = dpool.tile([TCH, DV], f32, name=f"dacca{b}", tag=f"dacca{b}")
            dacc_b = dpool.tile([TCH, DV], f32, name=f"daccb{b}", tag=f"daccb{b}")
            gacc = dpool.tile([TCH, DV], f32, name=f"gacc{b}", tag=f"gacc{b}")
            na = nb = ngp = 0
            afirst = bfirst = gfirst = None
            pe_tile = None
            for k in range(NCH):
                vt = vpool.tile([TCH, DV], f32, name=f"vt{b}_{k}", tag="vt")
                # alternate loads across both HWDGE rings (sync + scalar)
                eng = nc.sync if (b * NCH + k) % 2 == 0 else nc.scalar
                eng.dma_start(
                    out=vt[:], in_=vals[b, k * TCH:(k + 1) * TCH, :])
                if k == PE_CHUNK:
                    pe_tile = vt
                elif k in GP_CHUNKS:
                    ngp += 1
                    if ngp == 1:
                        gfirst = vt
                    elif ngp == 2:
                        nc.gpsimd.tensor_add(gacc[:], gfirst[:], vt[:])
                    else:
                        nc.gpsimd.tensor_add(gacc[:], gacc[:], vt[:])
                elif k in A_CHUNKS:
                    na += 1
                    if na == 1:
                        afirst = vt
                    elif na == 2:
                        nc.vector.tensor_add(dacc_a[:], afirst[:], vt[:])
                    else:
                        nc.vector.tensor_add(dacc_a[:], dacc_a[:], vt[:])
                else:
                    nb += 1
                    if nb == 1:
                        bfirst = vt
                    elif nb == 2:
                        nc.vector.tensor_add(dacc_b[:], bfirst[:], vt[:])
                    else:
                        nc.vector.tensor_add(dacc_b[:], dacc_b[:], vt[:])
                if k == GP_FOLD_AFTER:
                    nc.vector.tensor_add(dacc_a[:], dacc_a[:], gacc[:])
            ps = [
                ppool.tile([1, NSPLIT], f32, name=f"ps{b}_{j}", tag=f"ps{b}_{j}")
                for j in range(NJ)
            ]
            order = [dacc_a, dacc_b] if pe_tile is None else [dacc_a, dacc_b, pe_tile]
            for i, src in enumerate(order):
                for j in range(NJ):
                    sl = slice(j * NSPLIT, (j + 1) * NSPLIT)
                    nc.tensor.matmul(
                        ps[j][:], ones_t[:], src[:, sl],
                        start=(i == 0), stop=(i == len(order) - 1))

            ot = opool.tile([1, DV], f32, name=f"ot{b}", tag="ot")
            for j in range(NJ):
                nc.scalar.copy(ot[:, j * NSPLIT:(j + 1) * NSPLIT], ps[j][:])
            # context write on the scalar ring: keeps the sync HWDGE FIFO
            # free of output DMAs that would head-of-line block later loads
            nc.scalar.dma_start(out=ctx_out[b:b + 1, :], in_=ot[0:1, :])

    nc.compile()
    return nc


def kernel(query=None, values=None, **unused_weights):
    from concourse.bass_utils import run_bass_kernel_spmd

    values = np.ascontiguousarray(np.asarray(values, dtype=np.float32))
    assert values.shape == (B, T, DV), values.shape

    if "nc" not in _CACHE:
        _CACHE["nc"] = _build_nc()
    nc = _CACHE["nc"]

    consts = np.stack(
        [np.ones(T, dtype=np.float32), np.arange(T, dtype=np.float32)]
    )
    core_ids = list(range(NCORES))
    in_maps = [
        {"vals": values[c * BP:(c + 1) * BP], "consts": consts}
        for c in core_ids
    ]

    trace = bool(int(os.environ.get("BASS_KERNEL_TRACE", "0")))
    res = run_bass_kernel_spmd(nc, in_maps, core_ids, trace=trace)
    LAST["exec_time_ns"] = res.exec_time_ns
    LAST["results"] = res

    context = np.concatenate([res.results[c]["ctx_out"] for c in core_ids], axis=0)
    attn = np.concatenate([res.results[c]["attn_out"] for c in core_ids], axis=0)
    coverage = np.concatenate([res.results[c]["cov_out"] for c in core_ids], axis=0)
    return context, attn, coverage


# revision 41
# speedup vs baseline: 1.0195x; 1.0195x over previous
"""Bahdanau-attention kernel for Trainium2 (8 NeuronCores, SPMD data parallel).

Math: the reference's per-step softmax is over a singleton axis, so the
attention weights are exactly 1.0. Hence:
    context  = values.sum(axis=1)            [B, DV]
    attn     = ones(B, T, 1)
    coverage[b, t, 0] = t                    [B, T, 1]
The W1/W2/W3/V MLP cancels out of every output.

Device work: per core, reduce a [B/8, T, DV] shard of `values` over T.
All 64 chunk loads stream on the sync HWDGE ring (outputs go on the
scalar ring so they never head-of-line block the loads); the fp32 adds
are spread over three engines so none exceeds the HBM/DMA roofline:
  - DVE: two short tensor_add chains per batch (dacc_a, dacc_b)
  - GpSimd: 4 chunks per batch in its own chain (gacc), folded into
    dacc_a mid-stream
  - PE: contracts dacc_a/dacc_b over partitions vs a ones column into
    the per-batch PSUM accumulation group.
attn/coverage come from a tiny host const tensor, written out by DMA.
"""

import os
import numpy as np

B, T, DV = 32, 2048, 1024
NCORES = 8
BP = B // NCORES          # 4 batches per core
TCH = 128                 # t-chunk rows = SBUF partitions
NCH = T // TCH            # 16 chunks of [128, DV] per batch
NSPLIT = 512              # PSUM bank free-dim limit (f32)
NJ = DV // NSPLIT         # 2 psum column groups

# Engine roles (measured per-chunk costs: DVE add 1.22us, gpsimd add
# ~2.4us, PE direct contraction ~2.5us = 4 MM insts; gpsimd and DVE adds
# contend on SBUF ports, so gpsimd's share is kept small):
#   - gpsimd: 4 chunks per batch in its own chain (gacc)
#   - DVE: two short chains per batch (dacc_a over A_CHUNKS, dacc_b over
#     B_CHUNKS); gacc folds into dacc_a after chunk GP_FOLD_AFTER
#   - PE: only the two cross-partition contractions per batch
GP_CHUNKS = (1, 4, 7, 10)
A_CHUNKS = (0, 2, 3, 5, 6)
B_CHUNKS = (8, 9, 11, 12, 13, 14, 15)
PE_CHUNK = None
GP_FOLD_AFTER = 13        # gacc -> dacc_a fold point (gacc done by ~k13)

_CACHE = {}
LAST = {}                 # exec_time_ns etc. for the test harness


def _build_nc():
    import concourse.tile as tile
    from concourse import bacc, mybir
    from contextlib import ExitStack

    f32 = mybir.dt.float32
    nc = bacc.Bacc(
        "TRN2", target_bir_lowering=False, debug=False, num_devices=NCORES
    )

    vals = nc.dram_tensor("vals", [BP, T, DV], f32, kind="ExternalInput").ap()
    consts = nc.dram_tensor("consts", [2, T], f32, kind="ExternalInput").ap()
    ctx_out = nc.dram_tensor("ctx_out", [BP, DV], f32, kind="ExternalOutput").ap()
    attn_out = nc.dram_tensor("attn_out", [BP, T, 1], f32, kind="ExternalOutput").ap()
    cov_out = nc.dram_tensor("cov_out", [BP, T, 1], f32, kind="ExternalOutput").ap()

    with tile.TileContext(nc) as tc, ExitStack() as ctx:
        cpool = ctx.enter_context(tc.tile_pool(name="const", bufs=1))
        vpool = ctx.enter_context(tc.tile_pool(name="vals", bufs=24))
        dpool = ctx.enter_context(tc.tile_pool(name="dacc", bufs=1))
        ppool = ctx.enter_context(tc.tile_pool(name="ps", bufs=1, space="PSUM"))
        opool = ctx.enter_context(tc.tile_pool(name="out", bufs=2))

        ones_t = cpool.tile([128, 1], f32)
        nc.vector.memset(ones_t[:], 1.0)

        const_t = cpool.tile([2, T], f32)
        nc.sync.dma_start(out=const_t[:], in_=consts[:])

        # attn/coverage writes go on the scalar HWDGE ring, issued as soon
        # as const_t lands, so the sync ring carries only the big loads.
        for b in range(BP):
            nc.scalar.dma_start(out=attn_out[b:b + 1, :, 0], in_=const_t[0:1, :])
            nc.scalar.dma_start(out=cov_out[b:b + 1, :, 0], in_=const_t[1:2, :])

        for b in range(BP):
            dacc_a = dpool.tile([TCH, DV], f32, name=f"dacca{b}", tag=f"dacca{b}")
            dacc_b = dpool.tile([TCH, DV], f32, name=f"daccb{b}", tag=f"daccb{b}")
            gacc = dpool.tile([TCH, DV], f32, name=f"gacc{b}", tag=f"gacc{b}")
            na = nb = ngp = 0
            afirst = bfirst = gfirst = None
            pe_tile = None
            for k in range(NCH):
                vt = vpool.tile([TCH, DV], f32, name=f"vt{b}_{k}", tag="vt")
                nc.sync.dma_start(
                    out=vt[:], in_=vals[b, k * TCH:(k + 1) * TCH, :])
                if k == PE_CHUNK:
                    pe_tile = vt
                elif k in GP_CHUNKS:
                    ngp += 1
                    if ngp == 1:
                        gfirst = vt
                    elif ngp == 2:
                        nc.gpsimd.tensor_add(gacc[:], gfirst[:], vt[:])
                    else:
                        nc.gpsimd.tensor_add(gacc[:], gacc[:], vt[:])
                elif k in A_CHUNKS:
                    na += 1
                    if na == 1:
                        afirst = vt
                    elif na == 2:
                        nc.vector.tensor_add(dacc_a[:], afirst[:], vt[:])
                    else:
                        nc.vector.tensor_add(dacc_a[:], dacc_a[:], vt[:])
                else:
                    nb += 1
                    if nb == 1:
                        bfirst = vt
                    elif nb == 2:
                        nc.vector.tensor_add(dacc_b[:], bfirst[:], vt[:])
                    else:
                        nc.vector.tensor_add(dacc_b[:], dacc_b[:], vt[:])
                if k == GP_FOLD_AFTER:
                    nc.vector.tensor_add(dacc_a[:], dacc_a[:], gacc[:])
            ps = [
                ppool.tile([1, NSPLIT], f32, name=f"ps{b}_{j}", tag=f"ps{b}_{j}")
                for j in range(NJ)
            ]
            order = [dacc_a, dacc_b] if pe_tile is None else [dacc_a, dacc_b, pe_tile]
            for i, src in enumerate(order):
                for j in range(NJ):
                    sl = slice(j * NSPLIT, (j + 1) * NSPLIT)
                    nc.tensor.matmul(
                        ps[j][:], ones_t[:], src[:, sl],
                        start=(i == 0), stop=(i == len(order) - 1))

            ot = opool.tile([1, DV], f32, name=f"ot{b}", tag="ot")
            for j in range(NJ):
                nc.scalar.copy(ot[:, j * NSPLIT:(j + 1) * NSPLIT], ps[j][:])
            # context write on the scalar ring: keeps the sync HWDGE FIFO
            # free of output DMAs that would head-of-line block later loads
            nc.scalar.dma_start(out=ctx_out[b:b + 1, :], in_=ot[0:1, :])

    nc.compile()
    return nc


def kernel(query=None, values=None, **unused_weights):
    from concourse.bass_utils import run_bass_kernel_spmd

    values = np.ascontiguousarray(np.asarray(values, dtype=np.float32))
    assert values.shape == (B, T, DV), values.shape

    if "nc" not in _CACHE:
        _CACHE["nc"] = _build_nc()
    nc = _CACHE["nc"]

    consts = np.stack(
        [np.ones(T, dtype=np.float32), np.arange(T, dtype=np.float32)]
    )
    core_ids = list(range(NCORES))
    in_maps = [
        {"vals": values[c * BP:(c + 1) * BP], "consts": consts}
        for c in core_ids
    ]

    trace = bool(int(os.environ.get("BASS_KERNEL_TRACE", "0")))
    res = run_bass_kernel_spmd(nc, in_maps, core_ids, trace=trace)
    LAST["exec_time_ns"] = res.exec_time_ns
    LAST["results"] = res

    context = np.concatenate([res.results[c]["ctx_out"] for c in core_ids], axis=0)
    attn = np.concatenate([res.results[c]["attn_out"] for c in core_ids], axis=0)
    coverage = np.concatenate([res.results[c]["cov_out"] for c in core_ids], axis=0)
    return context, attn, coverage


# revision 43
# speedup vs baseline: 1.0459x; 1.0259x over previous
"""Bahdanau-attention kernel for Trainium2 (8 NeuronCores, SPMD data parallel).

Math: the reference's per-step softmax is over a singleton axis, so the
attention weights are exactly 1.0. Hence:
    context  = values.sum(axis=1)            [B, DV]
    attn     = ones(B, T, 1)
    coverage[b, t, 0] = t                    [B, T, 1]
The W1/W2/W3/V MLP cancels out of every output.

Device work: per core, reduce a [B/8, T, DV] shard of `values` over T.
All 64 chunk loads stream on the sync HWDGE ring (outputs go on the
scalar ring so they never head-of-line block the loads); the fp32 adds
are spread over three engines so none exceeds the HBM/DMA roofline:
  - DVE: two short tensor_add chains per batch (dacc_a, dacc_b)
  - GpSimd: 4 chunks per batch in its own chain (gacc), folded into
    dacc_a mid-stream
  - PE: contracts dacc_a/dacc_b over partitions vs a ones column into
    the per-batch PSUM accumulation group.
attn/coverage come from a tiny host const tensor, written out by DMA.
"""

import os
import numpy as np

B, T, DV = 32, 2048, 1024
NCORES = 8
BP = B // NCORES          # 4 batches per core
TCH = 128                 # t-chunk rows = SBUF partitions
NCH = T // TCH            # 16 chunks of [128, DV] per batch
NSPLIT = 512              # PSUM bank free-dim limit (f32)
NJ = DV // NSPLIT         # 2 psum column groups

# Engine roles (measured per-chunk costs: DVE add 1.22us, gpsimd add
# ~2.4us, PE direct contraction ~2.5us = 4 MM insts; gpsimd and DVE adds
# contend on SBUF ports, so gpsimd's share is kept small):
#   - gpsimd: 4 chunks per batch in its own chain (gacc)
#   - DVE: two short chains per batch (dacc_a over A_CHUNKS, dacc_b over
#     B_CHUNKS); gacc folds into dacc_a after chunk GP_FOLD_AFTER
#   - PE: only the two cross-partition contractions per batch
GP_CHUNKS = (1, 4, 7, 10)
A_CHUNKS = (0, 2, 3, 5, 6)
B_CHUNKS = (8, 9, 11, 12, 13, 14, 15)
PE_CHUNK = None
GP_FOLD_AFTER = 13        # gacc -> dacc_a fold point (gacc done by ~k13)
CROSSFADE = 4             # loads interleaved across each batch boundary


def _load_schedule():
    """Load order: batch-major, but each boundary crossfades the last
    CROSSFADE chunks of b with the first CROSSFADE of b+1, so the batch-end
    reduction convoy (gpsimd tail + fold + contraction) drains while fresh
    chunks with idle consumers keep freeing buffer slots."""
    order = []
    for b in range(BP):
        start = CROSSFADE if b > 0 else 0
        end = NCH - CROSSFADE if b < BP - 1 else NCH
        order.extend((b, k) for k in range(start, end))
        if b < BP - 1:
            for i in range(CROSSFADE):
                order.append((b, NCH - CROSSFADE + i))
                order.append((b + 1, i))
    assert sorted(order) == [(b, k) for b in range(BP) for k in range(NCH)]
    return order

_CACHE = {}
LAST = {}                 # exec_time_ns etc. for the test harness


def _build_nc():
    import concourse.tile as tile
    from concourse import bacc, mybir
    from contextlib import ExitStack

    f32 = mybir.dt.float32
    nc = bacc.Bacc(
        "TRN2", target_bir_lowering=False, debug=False, num_devices=NCORES
    )

    vals = nc.dram_tensor("vals", [BP, T, DV], f32, kind="ExternalInput").ap()
    consts = nc.dram_tensor("consts", [2, T], f32, kind="ExternalInput").ap()
    ctx_out = nc.dram_tensor("ctx_out", [BP, DV], f32, kind="ExternalOutput").ap()
    attn_out = nc.dram_tensor("attn_out", [BP, T, 1], f32, kind="ExternalOutput").ap()
    cov_out = nc.dram_tensor("cov_out", [BP, T, 1], f32, kind="ExternalOutput").ap()

    with tile.TileContext(nc) as tc, ExitStack() as ctx:
        cpool = ctx.enter_context(tc.tile_pool(name="const", bufs=1))
        vpool = ctx.enter_context(tc.tile_pool(name="vals", bufs=24))
        dpool = ctx.enter_context(tc.tile_pool(name="dacc", bufs=1))
        ppool = ctx.enter_context(tc.tile_pool(name="ps", bufs=1, space="PSUM"))
        opool = ctx.enter_context(tc.tile_pool(name="out", bufs=2))

        ones_t = cpool.tile([128, 1], f32)
        nc.vector.memset(ones_t[:], 1.0)

        const_t = cpool.tile([2, T], f32)
        nc.sync.dma_start(out=const_t[:], in_=consts[:])

        # attn/coverage writes go on the scalar HWDGE ring, issued as soon
        # as const_t lands, so the sync ring carries only the big loads.
        for b in range(BP):
            nc.scalar.dma_start(out=attn_out[b:b + 1, :, 0], in_=const_t[0:1, :])
            nc.scalar.dma_start(out=cov_out[b:b + 1, :, 0], in_=const_t[1:2, :])

        st = {}
        for b in range(BP):
            st[b] = {
                "dacc_a": dpool.tile(
                    [TCH, DV], f32, name=f"dacca{b}", tag=f"dacca{b}"),
                "dacc_b": dpool.tile(
                    [TCH, DV], f32, name=f"daccb{b}", tag=f"daccb{b}"),
                "gacc": dpool.tile(
                    [TCH, DV], f32, name=f"gacc{b}", tag=f"gacc{b}"),
                "na": 0, "nb": 0, "ngp": 0,
                "afirst": None, "bfirst": None, "gfirst": None,
                "pe_tile": None,
            }

        for b, k in _load_schedule():
            s = st[b]
            vt = vpool.tile([TCH, DV], f32, name=f"vt{b}_{k}", tag="vt")
            nc.sync.dma_start(
                out=vt[:], in_=vals[b, k * TCH:(k + 1) * TCH, :])
            if k == PE_CHUNK:
                s["pe_tile"] = vt
            elif k in GP_CHUNKS:
                s["ngp"] += 1
                if s["ngp"] == 1:
                    s["gfirst"] = vt
                elif s["ngp"] == 2:
                    nc.gpsimd.tensor_add(s["gacc"][:], s["gfirst"][:], vt[:])
                else:
                    nc.gpsimd.tensor_add(s["gacc"][:], s["gacc"][:], vt[:])
            elif k in A_CHUNKS:
                s["na"] += 1
                if s["na"] == 1:
                    s["afirst"] = vt
                elif s["na"] == 2:
                    nc.vector.tensor_add(s["dacc_a"][:], s["afirst"][:], vt[:])
                else:
                    nc.vector.tensor_add(s["dacc_a"][:], s["dacc_a"][:], vt[:])
            else:
                s["nb"] += 1
                if s["nb"] == 1:
                    s["bfirst"] = vt
                elif s["nb"] == 2:
                    nc.vector.tensor_add(s["dacc_b"][:], s["bfirst"][:], vt[:])
                else:
                    nc.vector.tensor_add(s["dacc_b"][:], s["dacc_b"][:], vt[:])
            if k == GP_FOLD_AFTER:
                nc.vector.tensor_add(s["dacc_a"][:], s["dacc_a"][:], s["gacc"][:])

            if k != NCH - 1:
                continue
            # b's final chunk scheduled: contract and write out
            ps = [
                ppool.tile([1, NSPLIT], f32, name=f"ps{b}_{j}", tag=f"ps{b}_{j}")
                for j in range(NJ)
            ]
            srcs = [s["dacc_a"], s["dacc_b"]]
            if s["pe_tile"] is not None:
                srcs.append(s["pe_tile"])
            for i, src in enumerate(srcs):
                for j in range(NJ):
                    sl = slice(j * NSPLIT, (j + 1) * NSPLIT)
                    nc.tensor.matmul(
                        ps[j][:], ones_t[:], src[:, sl],
                        start=(i == 0), stop=(i == len(srcs) - 1))

            ot = opool.tile([1, DV], f32, name=f"ot{b}", tag="ot")
            for j in range(NJ):
                nc.scalar.copy(ot[:, j * NSPLIT:(j + 1) * NSPLIT], ps[j][:])
            # context write on the scalar ring: keeps the sync HWDGE FIFO
            # free of output DMAs that would head-of-line block later loads
            nc.scalar.dma_start(out=ctx_out[b:b + 1, :], in_=ot[0:1, :])

    nc.compile()
    return nc


def kernel(query=None, values=None, **unused_weights):
    from concourse.bass_utils import run_bass_kernel_spmd

    values = np.ascontiguousarray(np.asarray(values, dtype=np.float32))
    assert values.shape == (B, T, DV), values.shape

    if "nc" not in _CACHE:
        _CACHE["nc"] = _build_nc()
    nc = _CACHE["nc"]

    consts = np.stack(
        [np.ones(T, dtype=np.float32), np.arange(T, dtype=np.float32)]
    )
    core_ids = list(range(NCORES))
    in_maps = [
        {"vals": values[c * BP:(c + 1) * BP], "consts": consts}
        for c in core_ids
    ]

    trace = bool(int(os.environ.get("BASS_KERNEL_TRACE", "0")))
    res = run_bass_kernel_spmd(nc, in_maps, core_ids, trace=trace)
    LAST["exec_time_ns"] = res.exec_time_ns
    LAST["results"] = res

    context = np.concatenate([res.results[c]["ctx_out"] for c in core_ids], axis=0)
    attn = np.concatenate([res.results[c]["attn_out"] for c in core_ids], axis=0)
    coverage = np.concatenate([res.results[c]["cov_out"] for c in core_ids], axis=0)
    return context, attn, coverage
